# revision 8
# baseline (speedup 1.0000x reference)
"""PointNet MLP (3 x conv1x1+BN+ReLU, final valid-mask) on 8 TRN2 cores.

Sharding: compacted-column parallel. The valid mask keeps ~70% of the
4096*128 = 524288 point-neighbor columns; masked columns are exactly 0 in
the reference output. Host gathers the valid columns, splits them evenly
across 8 cores, device computes only those, host scatters into zeros.

Numerics: single-term fp16 matmuls with f32 PSUM accumulation (end-to-end
rel err ~1e-3 vs the 2e-2 gate). BN folded into conv weights/bias on host.

Device per-core loop (iters x 1024 columns, block-pair A|B of 512):
 - L1 (3->64): one K=6 matmul, block-diag lhsT maps xA rows 0:3 -> out
   channels 0:64 and xB rows 3:6 -> 64:128. relu+b1 on ACT -> hi1 f16.
 - L2 (64->64): one K=128 block-diag matmul. relu+b2 on ACT -> hi2 f16.
 - L3 (64->128): two concurrent row-tiled K=64 matmuls (array rows 0:63
   for block A, 64:127 for block B) into adjacent PSUM banks.
   relu+b3 on DVE tensor_scalar -> f16 -> DMA out.
Tile's scheduler software-pipelines adjacent iterations on its own; PSUM
is fully double-buffered (2+2+4 banks).

Startup optimizations:
 - xp input lands on SBUF partitions 0:6 = one SDMA engine (~27 GiB/s),
   so it is loaded in 6 chunks to unblock iteration 0 after ~1/6 of it.
 - A dummy 1-element activation triggers the ~1.3us ACT table load while
   the input DMAs run.
 - The PE HAM clock gate only opens (1.2 -> 2.4 GHz) after ~3.4us of
   gapless matmul activity: a 12-matmul warmup burst on scratch data runs
   under the input DMAs, and filler matmuls in the first 3 iterations keep
   the PE dense through the pipeline ramp so it never re-throttles (the
   steady-state loop is back-to-back on the PE and stays warm).
"""

import numpy as np

try:
    import concourse.bass as bass
except ImportError:
    import sys

    sys.path.insert(0, "/opt/trn_rl_repo")
    import concourse.bass as bass

import concourse.bacc as bacc

import concourse.mybir as mybir
from concourse import tile
from concourse.bass_utils import run_bass_kernel_spmd

F32 = mybir.dt.float32
F16 = mybir.dt.float16

N_CORES = 8
NPOINT, KNN = 4096, 128
NCOLS = NPOINT * KNN
M = 512
EPS = 1e-5
N_WARM = 9
N_CHUNK = 12
N_FILL = 3

_NC_CACHE = {}


def _build_nc(iters):
    nc = bacc.Bacc("TRN2", target_bir_lowering=False)
    xp_d = nc.declare_dram_parameter("xp", [128, iters * M], F16, isOutput=False)
    w1_d = nc.declare_dram_parameter("lhsT1", [128, 128], F16, isOutput=False)
    w2_d = nc.declare_dram_parameter("lhsT2", [128, 128], F16, isOutput=False)
    w3a_d = nc.declare_dram_parameter("lhsT3a", [128, 128], F16, isOutput=False)
    w3b_d = nc.declare_dram_parameter("lhsT3b", [128, 128], F16, isOutput=False)
    bias_d = nc.declare_dram_parameter("biases", [128, 3], F32, isOutput=False)
    out_d = nc.declare_dram_parameter("out", [128, iters * 2 * M], F16, isOutput=True)

    add = mybir.AluOpType.add
    vmax = mybir.AluOpType.max
    relu_fn = mybir.ActivationFunctionType.Relu

    with tile.TileContext(nc) as tc:
        with (
            tc.tile_pool(name="const", bufs=1) as cpool,
            tc.tile_pool(name="xpool", bufs=1) as xpool,
            tc.tile_pool(name="ypool", bufs=3) as ypool,
            tc.tile_pool(name="opool", bufs=3) as opool,
            tc.tile_pool(name="pspool", bufs=2, space="PSUM") as pspool,
        ):
            scratch = cpool.tile([128, 512], F16, tag="scratch")
            nc.vector.memset(scratch[:, :], 0)
            # First ACTIVATE in program order triggers the ~1.3us ACT table
            # load; run it on scratch so it overlaps the input DMAs.
            nc.scalar.activation(scratch[0:1, 480:481], scratch[0:1, 0:1],
                                 relu_fn)

            w1_sb = cpool.tile([128, 128], F16, tag="w1")
            w2_sb = cpool.tile([128, 128], F16, tag="w2")
            w3a_sb = cpool.tile([128, 128], F16, tag="w3a")
            w3b_sb = cpool.tile([128, 128], F16, tag="w3b")
            bias_sb = cpool.tile([128, 3], F32, tag="bias")
            x_sb = xpool.tile([128, iters * M], F16, tag="x")

            nc.sync.dma_start(w1_sb[:, :], w1_d[:, :])
            nc.sync.dma_start(bias_sb[:, :], bias_d[:, :])
            csz = -(-iters // N_CHUNK) * M
            for c in range(0, iters * M, csz):
                ce = min(c + csz, iters * M)
                nc.sync.dma_start(x_sb[:, c:ce], xp_d[:, c:ce])
            nc.sync.dma_start(w2_sb[:, :], w2_d[:, :])
            nc.sync.dma_start(w3a_sb[:, :], w3a_d[:, :])
            nc.sync.dma_start(w3b_sb[:, :], w3b_d[:, :])

            b1_ap = bias_sb[:, 0:1]
            b2_ap = bias_sb[:, 1:2]
            b3_ap = bias_sb[:, 2:3]

            # Gapless warmup matmuls to open the PE HAM clock gate before
            # the real matmuls start; alternating weight slices let each
            # LDWEIGHTS overlap the previous matmul.
            for w in range(N_WARM):
                wps = pspool.tile([128, M], F32, tag="ps1", name=f"warm{w}")
                wsl = scratch[:, 128:256] if w % 2 else scratch[:, 0:128]
                nc.tensor.matmul(wps[:, :], wsl, scratch[:, :])

            for j in range(iters):
                c0, c1 = j * M, (j + 1) * M
                ps1 = pspool.tile([128, M], F32, tag="ps1", name="ps1")
                ps2 = pspool.tile([128, M], F32, tag="ps2", name="ps2")
                ps3 = pspool.tile([128, 2 * M], F32, tag="ps3", name="ps3")
                hi1 = ypool.tile([128, M], F16, tag="hi1", name="hi1")
                hi2 = ypool.tile([128, M], F16, tag="hi2", name="hi2")
                ob = opool.tile([128, 2 * M], F16, tag="ob", name="ob")

                if j < N_FILL:
                    # Keep the PE dense through the pipeline ramp so the HAM
                    # stays at 8/8; overwritten by the real matmul below.
                    nc.tensor.matmul(ps1[:, :], scratch[:, 0:128],
                                     scratch[:, :])
                nc.tensor.matmul(ps1[:, :], w1_sb[:, :], x_sb[:, c0:c1])
                nc.scalar.activation(hi1[:, :], ps1[:, :], relu_fn,
                                     bias=b1_ap)

                nc.tensor.matmul(ps2[:, :], w2_sb[:, :], hi1[:, :])
                nc.scalar.activation(hi2[:, :], ps2[:, :], relu_fn,
                                     bias=b2_ap)

                # Filler matmul on scratch: fills the PE idle slot while
                # waiting on relu2, keeping the HAM clock gate at 8/8 for
                # the whole loop (a warm PE idling ~25% per iteration gets
                # re-throttled to 1.2 GHz). Overwritten by the real matmul.
                nc.tensor.matmul(ps3[:, 0:M], scratch[:, 0:128],
                                 scratch[:, :])
                nc.tensor.matmul(ps3[:, 0:M], w3a_sb[:, :], hi2[:, :])
                nc.tensor.matmul(ps3[:, M : 2 * M], w3b_sb[:, :], hi2[:, :])
                nc.vector.tensor_scalar(ob[:, :], ps3[:, :], b3_ap, 0.0,
                                        add, vmax)
                nc.sync.dma_start(out_d[:, 2 * c0 : 2 * c1], ob[:, :])

    nc.compile()
    return nc


def _get_nc(iters):
    if iters not in _NC_CACHE:
        _NC_CACHE[iters] = _build_nc(iters)
    return _NC_CACHE[iters]


def _fold_bn(W, b, gamma, beta, mean, var):
    inv = gamma.astype(np.float64) / np.sqrt(var.astype(np.float64) + EPS)
    Wp = (W.astype(np.float64) * inv[:, None]).astype(np.float32)
    bp = ((b.astype(np.float64) - mean.astype(np.float64)) * inv
          + beta.astype(np.float64)).astype(np.float32)
    return Wp, bp


def _prepare(inputs):
    gp = np.asarray(inputs["grouped_pc"], dtype=np.float32)
    valid = np.asarray(inputs["valid"], dtype=np.float32)

    Wp1, bp1 = _fold_bn(*(np.asarray(inputs[k], dtype=np.float32)
                          for k in ("W1", "b1", "gamma1", "beta1", "mean1", "var1")))
    Wp2, bp2 = _fold_bn(*(np.asarray(inputs[k], dtype=np.float32)
                          for k in ("W2", "b2", "gamma2", "beta2", "mean2", "var2")))
    Wp3, bp3 = _fold_bn(*(np.asarray(inputs[k], dtype=np.float32)
                          for k in ("W3", "b3", "gamma3", "beta3", "mean3", "var3")))

    lhsT1 = np.zeros((128, 128), np.float16)
    lhsT1[0:3, 0:64] = Wp1.T
    lhsT1[3:6, 64:128] = Wp1.T

    lhsT2 = np.zeros((128, 128), np.float16)
    lhsT2[0:64, 0:64] = Wp2.T
    lhsT2[64:128, 64:128] = Wp2.T

    lhsT3a = np.zeros((128, 128), np.float16)
    lhsT3a[0:64, :] = Wp3.T
    lhsT3b = np.zeros((128, 128), np.float16)
    lhsT3b[64:128, :] = Wp3.T

    biases = np.zeros((128, 3), np.float32)
    biases[:, 0] = np.concatenate([bp1, bp1])
    biases[:, 1] = np.concatenate([bp2, bp2])
    biases[:, 2] = bp3

    x = gp[0].reshape(3, NCOLS)
    vidx = np.flatnonzero(valid.reshape(NCOLS) > 0.5)
    V = len(vidx)
    Vc = -(-V // N_CORES)
    iters = max(1, -(-Vc // (2 * M)))
    cap = iters * 2 * M

    xv = x[:, vidx].astype(np.float16)

    in_maps = []
    for c in range(N_CORES):
        lo_i = c * Vc
        hi_i = min((c + 1) * Vc, V)
        n = max(0, hi_i - lo_i)
        a = np.zeros((3, cap), np.float16)
        if n:
            a[:, :n] = xv[:, lo_i:hi_i]
        ar = a.reshape(3, iters, 2, M)
        xp = np.zeros((128, iters * M), np.float16)
        xp[0:3] = ar[:, :, 0, :].reshape(3, -1)
        xp[3:6] = ar[:, :, 1, :].reshape(3, -1)
        in_maps.append(
            {
                "xp": np.ascontiguousarray(xp),
                "lhsT1": lhsT1,
                "lhsT2": lhsT2,
                "lhsT3a": lhsT3a,
                "lhsT3b": lhsT3b,
                "biases": biases,
            }
        )
    return in_maps, vidx, V, Vc, iters


def _gather(results, vidx, V, Vc):
    stream = np.empty((128, V), np.float32)
    for c in range(N_CORES):
        lo_i = c * Vc
        hi_i = min((c + 1) * Vc, V)
        if hi_i <= lo_i:
            break
        stream[:, lo_i:hi_i] = results[c]["out"][:, : hi_i - lo_i]
    full = np.zeros((128, NCOLS), np.float32)
    full[:, vidx] = stream
    return full.reshape(128, NPOINT, KNN)[None]


def run_traced(trace=False, **inputs):
    in_maps, vidx, V, Vc, iters = _prepare(inputs)
    nc = _get_nc(iters)
    res = run_bass_kernel_spmd(nc, in_maps, list(range(N_CORES)), trace=trace)
    return _gather(res.results, vidx, V, Vc), res.exec_time_ns


def kernel(**inputs):
    out, _ = run_traced(trace=False, **inputs)
    return out


# revision 10
# speedup vs baseline: 1.1003x; 1.1003x over previous
"""PointNet MLP (3 x conv1x1+BN+ReLU, final valid-mask) on 8 TRN2 cores.

Sharding: compacted-column parallel. The valid mask keeps ~70% of the
4096*128 = 524288 point-neighbor columns; masked columns are exactly 0 in
the reference output. Host gathers the valid columns, splits them evenly
across 8 cores, device computes only those, host scatters into zeros.

Numerics: single-term fp16 matmuls with f32 PSUM accumulation (end-to-end
rel err ~1e-3 vs the 2e-2 gate). BN folded into conv weights/bias on host.

Device per-core loop (iters x 1024 columns, block-pair A|B of 512):
 - L1 (3->64): one K=6 matmul, block-diag lhsT maps xA rows 0:3 -> out
   channels 0:64 and xB rows 3:6 -> 64:128. relu+b1 on ACT -> hi1 f16.
 - L2 (64->64): one K=128 block-diag matmul. relu+b2 on ACT -> hi2 f16.
 - L3 (64->128): two concurrent row-tiled K=64 matmuls (array rows 0:63
   for block A, 64:127 for block B) into adjacent PSUM banks.
   relu+b3 on DVE tensor_scalar -> f16 -> DMA out.
Tile's scheduler software-pipelines adjacent iterations on its own; PSUM
is fully double-buffered (2+2+4 banks).

Startup optimizations:
 - xp input lands on SBUF partitions 0:6 = one SDMA engine (~27 GiB/s),
   so it is loaded in 6 chunks to unblock iteration 0 after ~1/6 of it.
 - A dummy 1-element activation triggers the ~1.3us ACT table load while
   the input DMAs run.
 - The PE HAM clock gate only opens (1.2 -> 2.4 GHz) after ~3.4us of
   gapless matmul activity: a 12-matmul warmup burst on scratch data runs
   under the input DMAs, and filler matmuls in the first 3 iterations keep
   the PE dense through the pipeline ramp so it never re-throttles (the
   steady-state loop is back-to-back on the PE and stays warm).
"""

import numpy as np

try:
    import concourse.bass as bass
except ImportError:
    import sys

    sys.path.insert(0, "/opt/trn_rl_repo")
    import concourse.bass as bass

import concourse.bacc as bacc

import concourse.mybir as mybir
from concourse import tile
from concourse.bass_utils import run_bass_kernel_spmd

F32 = mybir.dt.float32
F16 = mybir.dt.float16

N_CORES = 8
NPOINT, KNN = 4096, 128
NCOLS = NPOINT * KNN
M = 512
EPS = 1e-5
N_WARM = 9
N_CHUNK = 6
N_FILL = 3

_NC_CACHE = {}


def _build_nc(iters):
    nc = bacc.Bacc("TRN2", target_bir_lowering=False)
    xp_d = nc.declare_dram_parameter("xp", [6, iters * M], F16, isOutput=False)
    w1_d = nc.declare_dram_parameter("lhsT1", [128, 128], F16, isOutput=False)
    w2_d = nc.declare_dram_parameter("lhsT2", [128, 128], F16, isOutput=False)
    w3a_d = nc.declare_dram_parameter("lhsT3a", [128, 128], F16, isOutput=False)
    w3b_d = nc.declare_dram_parameter("lhsT3b", [128, 128], F16, isOutput=False)
    bias_d = nc.declare_dram_parameter("biases", [128, 3], F32, isOutput=False)
    out_d = nc.declare_dram_parameter("out", [128, iters * 2 * M], F16, isOutput=True)

    add = mybir.AluOpType.add
    vmax = mybir.AluOpType.max
    relu_fn = mybir.ActivationFunctionType.Relu

    with tile.TileContext(nc) as tc:
        with (
            tc.tile_pool(name="const", bufs=1) as cpool,
            tc.tile_pool(name="xpool", bufs=1) as xpool,
            tc.tile_pool(name="ypool", bufs=3) as ypool,
            tc.tile_pool(name="opool", bufs=3) as opool,
            tc.tile_pool(name="pspool", bufs=2, space="PSUM") as pspool,
        ):
            scratch = cpool.tile([128, 512], F16, tag="scratch")
            nc.vector.memset(scratch[:, :], 0)
            # First ACTIVATE in program order triggers the ~1.3us ACT table
            # load; run it on scratch so it overlaps the input DMAs.
            nc.scalar.activation(scratch[0:1, 480:481], scratch[0:1, 0:1],
                                 relu_fn)

            w1_sb = cpool.tile([128, 128], F16, tag="w1")
            w2_sb = cpool.tile([128, 128], F16, tag="w2")
            w3a_sb = cpool.tile([128, 128], F16, tag="w3a")
            w3b_sb = cpool.tile([128, 128], F16, tag="w3b")
            bias_sb = cpool.tile([128, 3], F32, tag="bias")
            x_sb = xpool.tile([128, iters * M], F16, tag="x")

            nc.sync.dma_start(w1_sb[:, :], w1_d[:, :])
            nc.sync.dma_start(bias_sb[:, :], bias_d[:, :])
            # xp holds only the 6 real rows (it lands on SBUF partitions
            # 0:6 = one SDMA engine, so it is chunked). The zero padding
            # that makes MM1 a full 128x128-config matmul is written by the
            # otherwise-idle GpSimd engine: BIR only accepts full-partition
            # memsets, so each chunk is zeroed across all 128 partitions
            # first and the DMA then overwrites rows 0:6. Graduated chunk
            # sizes unblock iteration 0 early.
            bounds, pos = [0], 0
            for w in (2, 3, 4, 6, 8, 8, 8):
                pos = min(pos + w, iters)
                if bounds[-1] != pos:
                    bounds.append(pos)
            if bounds[-1] != iters:
                bounds.append(iters)
            for b0, b1 in zip(bounds[:-1], bounds[1:]):
                c, ce = b0 * M, b1 * M
                nc.gpsimd.memset(x_sb[:, c:ce], 0)
                nc.sync.dma_start(x_sb[0:6, c:ce], xp_d[:, c:ce])
            nc.sync.dma_start(w2_sb[:, :], w2_d[:, :])
            nc.sync.dma_start(w3a_sb[:, :], w3a_d[:, :])
            nc.sync.dma_start(w3b_sb[:, :], w3b_d[:, :])

            b1_ap = bias_sb[:, 0:1]
            b2_ap = bias_sb[:, 1:2]
            b3_ap = bias_sb[:, 2:3]

            # Gapless warmup matmuls to open the PE HAM clock gate before
            # the real matmuls start; alternating weight slices let each
            # LDWEIGHTS overlap the previous matmul.
            for w in range(N_WARM):
                wps = pspool.tile([128, M], F32, tag="ps1", name=f"warm{w}")
                wsl = scratch[:, 128:256] if w % 2 else scratch[:, 0:128]
                nc.tensor.matmul(wps[:, :], wsl, scratch[:, :])

            for j in range(iters):
                c0, c1 = j * M, (j + 1) * M
                ps1 = pspool.tile([128, M], F32, tag="ps1", name="ps1")
                ps2 = pspool.tile([128, M], F32, tag="ps2", name="ps2")
                ps3 = pspool.tile([128, 2 * M], F32, tag="ps3", name="ps3")
                hi1 = ypool.tile([128, M], F16, tag="hi1", name="hi1")
                hi2 = ypool.tile([128, M], F16, tag="hi2", name="hi2")
                ob = opool.tile([128, 2 * M], F16, tag="ob", name="ob")

                if j < N_FILL:
                    # Keep the PE dense through the pipeline ramp so the HAM
                    # stays at 8/8; overwritten by the real matmul below.
                    nc.tensor.matmul(ps1[:, :], scratch[:, 0:128],
                                     scratch[:, :])
                nc.tensor.matmul(ps1[:, :], w1_sb[:, :], x_sb[:, c0:c1])
                nc.scalar.activation(hi1[:, :], ps1[:, :], relu_fn,
                                     bias=b1_ap)

                nc.tensor.matmul(ps2[:, :], w2_sb[:, :], hi1[:, :])
                nc.scalar.activation(hi2[:, :], ps2[:, :], relu_fn,
                                     bias=b2_ap)

                # Filler matmul on scratch: fills the PE idle slot while
                # waiting on relu2, keeping the HAM clock gate at 8/8 for
                # the whole loop (a warm PE idling ~25% per iteration gets
                # re-throttled to 1.2 GHz). Overwritten by the real matmul.
                nc.tensor.matmul(ps3[:, 0:M], scratch[:, 0:128],
                                 scratch[:, :])
                nc.tensor.matmul(ps3[:, 0:M], w3a_sb[:, :], hi2[:, :])
                nc.tensor.matmul(ps3[:, M : 2 * M], w3b_sb[:, :], hi2[:, :])
                nc.vector.tensor_scalar(ob[:, :], ps3[:, :], b3_ap, 0.0,
                                        add, vmax)
                nc.sync.dma_start(out_d[:, 2 * c0 : 2 * c1], ob[:, :])

    nc.compile()
    return nc


def _get_nc(iters):
    if iters not in _NC_CACHE:
        _NC_CACHE[iters] = _build_nc(iters)
    return _NC_CACHE[iters]


def _fold_bn(W, b, gamma, beta, mean, var):
    inv = gamma.astype(np.float64) / np.sqrt(var.astype(np.float64) + EPS)
    Wp = (W.astype(np.float64) * inv[:, None]).astype(np.float32)
    bp = ((b.astype(np.float64) - mean.astype(np.float64)) * inv
          + beta.astype(np.float64)).astype(np.float32)
    return Wp, bp


def _prepare(inputs):
    gp = np.asarray(inputs["grouped_pc"], dtype=np.float32)
    valid = np.asarray(inputs["valid"], dtype=np.float32)

    Wp1, bp1 = _fold_bn(*(np.asarray(inputs[k], dtype=np.float32)
                          for k in ("W1", "b1", "gamma1", "beta1", "mean1", "var1")))
    Wp2, bp2 = _fold_bn(*(np.asarray(inputs[k], dtype=np.float32)
                          for k in ("W2", "b2", "gamma2", "beta2", "mean2", "var2")))
    Wp3, bp3 = _fold_bn(*(np.asarray(inputs[k], dtype=np.float32)
                          for k in ("W3", "b3", "gamma3", "beta3", "mean3", "var3")))

    lhsT1 = np.zeros((128, 128), np.float16)
    lhsT1[0:3, 0:64] = Wp1.T
    lhsT1[3:6, 64:128] = Wp1.T

    lhsT2 = np.zeros((128, 128), np.float16)
    lhsT2[0:64, 0:64] = Wp2.T
    lhsT2[64:128, 64:128] = Wp2.T

    lhsT3a = np.zeros((128, 128), np.float16)
    lhsT3a[0:64, :] = Wp3.T
    lhsT3b = np.zeros((128, 128), np.float16)
    lhsT3b[64:128, :] = Wp3.T

    biases = np.zeros((128, 3), np.float32)
    biases[:, 0] = np.concatenate([bp1, bp1])
    biases[:, 1] = np.concatenate([bp2, bp2])
    biases[:, 2] = bp3

    x = gp[0].reshape(3, NCOLS)
    vidx = np.flatnonzero(valid.reshape(NCOLS) > 0.5)
    V = len(vidx)
    Vc = -(-V // N_CORES)
    iters = max(1, -(-Vc // (2 * M)))
    cap = iters * 2 * M

    xv = x[:, vidx].astype(np.float16)

    in_maps = []
    for c in range(N_CORES):
        lo_i = c * Vc
        hi_i = min((c + 1) * Vc, V)
        n = max(0, hi_i - lo_i)
        a = np.zeros((3, cap), np.float16)
        if n:
            a[:, :n] = xv[:, lo_i:hi_i]
        ar = a.reshape(3, iters, 2, M)
        xp = np.empty((6, iters * M), np.float16)
        xp[0:3] = ar[:, :, 0, :].reshape(3, -1)
        xp[3:6] = ar[:, :, 1, :].reshape(3, -1)
        in_maps.append(
            {
                "xp": np.ascontiguousarray(xp),
                "lhsT1": lhsT1,
                "lhsT2": lhsT2,
                "lhsT3a": lhsT3a,
                "lhsT3b": lhsT3b,
                "biases": biases,
            }
        )
    return in_maps, vidx, V, Vc, iters


def _gather(results, vidx, V, Vc):
    stream = np.empty((128, V), np.float32)
    for c in range(N_CORES):
        lo_i = c * Vc
        hi_i = min((c + 1) * Vc, V)
        if hi_i <= lo_i:
            break
        stream[:, lo_i:hi_i] = results[c]["out"][:, : hi_i - lo_i]
    full = np.zeros((128, NCOLS), np.float32)
    full[:, vidx] = stream
    return full.reshape(128, NPOINT, KNN)[None]


def run_traced(trace=False, **inputs):
    in_maps, vidx, V, Vc, iters = _prepare(inputs)
    nc = _get_nc(iters)
    res = run_bass_kernel_spmd(nc, in_maps, list(range(N_CORES)), trace=trace)
    return _gather(res.results, vidx, V, Vc), res.exec_time_ns


def kernel(**inputs):
    out, _ = run_traced(trace=False, **inputs)
    return out


# revision 12
# speedup vs baseline: 1.1659x; 1.0596x over previous
"""PointNet MLP (3 x conv1x1+BN+ReLU, final valid-mask) on 8 TRN2 cores.

Sharding: compacted-column parallel. The valid mask keeps ~70% of the
4096*128 = 524288 point-neighbor columns; masked columns are exactly 0 in
the reference output. Host gathers the valid columns, splits them evenly
across 8 cores, device computes only those, host scatters into zeros.

Numerics: single-term fp16 matmuls with f32 PSUM accumulation (end-to-end
rel err ~1e-3 vs the 2e-2 gate). BN folded into conv weights/bias on host.

Device per-core loop (iters x 1024 columns, block-pair A|B of 512):
 - L1 (3->64): one K=6 matmul, block-diag lhsT maps xA rows 0:3 -> out
   channels 0:64 and xB rows 3:6 -> 64:128. relu+b1 on ACT -> hi1 f16.
 - L2 (64->64): one K=128 block-diag matmul. relu+b2 on ACT -> hi2 f16.
 - L3 (64->128): two concurrent row-tiled K=64 matmuls (array rows 0:63
   for block A, 64:127 for block B) into adjacent PSUM banks.
   relu+b3 on DVE tensor_scalar -> f16 -> DMA out.
Tile's scheduler software-pipelines adjacent iterations on its own; PSUM
is fully double-buffered (2+2+4 banks).

Startup optimizations:
 - xp input lands on SBUF partitions 0:6 = one SDMA engine (~27 GiB/s),
   so it is loaded in 6 chunks to unblock iteration 0 after ~1/6 of it.
 - A dummy 1-element activation triggers the ~1.3us ACT table load while
   the input DMAs run.
 - The PE HAM clock gate only opens (1.2 -> 2.4 GHz) after ~3.4us of
   gapless matmul activity: a 12-matmul warmup burst on scratch data runs
   under the input DMAs, and filler matmuls in the first 3 iterations keep
   the PE dense through the pipeline ramp so it never re-throttles (the
   steady-state loop is back-to-back on the PE and stays warm).
"""

import numpy as np

try:
    import concourse.bass as bass
except ImportError:
    import sys

    sys.path.insert(0, "/opt/trn_rl_repo")
    import concourse.bass as bass

import concourse.bacc as bacc

import concourse.mybir as mybir
from concourse import tile
from concourse.bass_utils import run_bass_kernel_spmd

F32 = mybir.dt.float32
F16 = mybir.dt.float16

N_CORES = 8
NPOINT, KNN = 4096, 128
NCOLS = NPOINT * KNN
M = 512
EPS = 1e-5
N_WARM = 9
N_CHUNK = 6
N_FILL = 3

_NC_CACHE = {}


def _build_nc(iters):
    nc = bacc.Bacc("TRN2", target_bir_lowering=False)
    xp_d = nc.declare_dram_parameter("xp", [6, iters * M], F16, isOutput=False)
    w1_d = nc.declare_dram_parameter("lhsT1", [128, 128], F16, isOutput=False)
    w2_d = nc.declare_dram_parameter("lhsT2", [128, 128], F16, isOutput=False)
    w3a_d = nc.declare_dram_parameter("lhsT3a", [128, 128], F16, isOutput=False)
    w3b_d = nc.declare_dram_parameter("lhsT3b", [128, 128], F16, isOutput=False)
    bias_d = nc.declare_dram_parameter("biases", [128, 3], F32, isOutput=False)
    out_d = nc.declare_dram_parameter("out", [128, iters * 2 * M], F16, isOutput=True)

    add = mybir.AluOpType.add
    vmax = mybir.AluOpType.max
    relu_fn = mybir.ActivationFunctionType.Relu

    with tile.TileContext(nc) as tc:
        with (
            tc.tile_pool(name="const", bufs=1) as cpool,
            tc.tile_pool(name="xpool", bufs=1) as xpool,
            tc.tile_pool(name="ypool", bufs=3) as ypool,
            tc.tile_pool(name="opool", bufs=3) as opool,
            tc.tile_pool(name="pspool", bufs=2, space="PSUM") as pspool,
        ):
            scratch = cpool.tile([128, 512], F16, tag="scratch")
            nc.vector.memset(scratch[:, :], 0)
            # First ACTIVATE in program order triggers the ~1.3us ACT table
            # load; run it on scratch so it overlaps the input DMAs.
            nc.scalar.activation(scratch[0:1, 480:481], scratch[0:1, 0:1],
                                 relu_fn)

            w1_sb = cpool.tile([128, 128], F16, tag="w1")
            w2_sb = cpool.tile([128, 128], F16, tag="w2")
            w3a_sb = cpool.tile([128, 128], F16, tag="w3a")
            w3b_sb = cpool.tile([128, 128], F16, tag="w3b")
            bias_sb = cpool.tile([128, 3], F32, tag="bias")
            x_sb = xpool.tile([128, iters * M], F16, tag="x")

            nc.sync.dma_start(w1_sb[:, :], w1_d[:, :])
            nc.sync.dma_start(bias_sb[:, :], bias_d[:, :])
            # xp holds only the 6 real rows (it lands on SBUF partitions
            # 0:6 = one SDMA engine, so it is chunked). The zero padding
            # that makes MM1 a full 128x128-config matmul is written by the
            # otherwise-idle GpSimd engine: BIR only accepts full-partition
            # memsets, so each chunk is zeroed across all 128 partitions
            # first and the DMA then overwrites rows 0:6. Graduated chunk
            # sizes unblock iteration 0 early.
            bounds, pos = [0], 0
            for w in (2, 3, 4, 6, 8, 8, 8):
                pos = min(pos + w, iters)
                if bounds[-1] != pos:
                    bounds.append(pos)
            if bounds[-1] != iters:
                bounds.append(iters)
            chunks = list(zip(bounds[:-1], bounds[1:]))
            for b0, b1 in chunks:
                nc.gpsimd.memset(x_sb[:, b0 * M : b1 * M], 0)
            # Chunks 0-1 load upfront; later chunk DMAs are emitted from
            # inside the loop a few iterations ahead of use, so a chunk DMA
            # still waiting on its memset never head-of-line-blocks the
            # out-DMAs behind it on the in-order sync queue.
            dma_at = {}
            for k, (b0, b1) in enumerate(chunks):
                if k < 2:
                    nc.sync.dma_start(x_sb[0:6, b0 * M : b1 * M],
                                      xp_d[:, b0 * M : b1 * M])
                else:
                    dma_at[max(0, b0 - 3)] = (b0, b1)
            nc.sync.dma_start(w2_sb[:, :], w2_d[:, :])
            nc.sync.dma_start(w3a_sb[:, :], w3a_d[:, :])
            nc.sync.dma_start(w3b_sb[:, :], w3b_d[:, :])

            b1_ap = bias_sb[:, 0:1]
            b2_ap = bias_sb[:, 1:2]
            b3_ap = bias_sb[:, 2:3]

            # Gapless warmup matmuls to open the PE HAM clock gate before
            # the real matmuls start; alternating weight slices let each
            # LDWEIGHTS overlap the previous matmul.
            for w in range(N_WARM):
                wps = pspool.tile([128, M], F32, tag="ps1", name=f"warm{w}")
                wsl = scratch[:, 128:256] if w % 2 else scratch[:, 0:128]
                nc.tensor.matmul(wps[:, :], wsl, scratch[:, :])

            for j in range(iters):
                if j in dma_at:
                    b0, b1 = dma_at[j]
                    nc.sync.dma_start(x_sb[0:6, b0 * M : b1 * M],
                                      xp_d[:, b0 * M : b1 * M])
                c0, c1 = j * M, (j + 1) * M
                ps1 = pspool.tile([128, M], F32, tag="ps1", name="ps1")
                ps2 = pspool.tile([128, M], F32, tag="ps2", name="ps2")
                ps3 = pspool.tile([128, 2 * M], F32, tag="ps3", name="ps3")
                hi1 = ypool.tile([128, M], F16, tag="hi1", name="hi1")
                hi2 = ypool.tile([128, M], F16, tag="hi2", name="hi2")
                ob = opool.tile([128, 2 * M], F16, tag="ob", name="ob")

                if j < N_FILL:
                    # Keep the PE dense through the pipeline ramp so the HAM
                    # stays at 8/8; overwritten by the real matmul below.
                    nc.tensor.matmul(ps1[:, :], scratch[:, 0:128],
                                     scratch[:, :])
                nc.tensor.matmul(ps1[:, :], w1_sb[:, :], x_sb[:, c0:c1])
                nc.scalar.activation(hi1[:, :], ps1[:, :], relu_fn,
                                     bias=b1_ap)

                nc.tensor.matmul(ps2[:, :], w2_sb[:, :], hi1[:, :])
                nc.scalar.activation(hi2[:, :], ps2[:, :], relu_fn,
                                     bias=b2_ap)

                # Filler matmul on scratch: fills the PE idle slot while
                # waiting on relu2, keeping the HAM clock gate at 8/8 for
                # the whole loop (a warm PE idling ~25% per iteration gets
                # re-throttled to 1.2 GHz). Overwritten by the real matmul.
                nc.tensor.matmul(ps3[:, 0:M], scratch[:, 0:128],
                                 scratch[:, :])
                nc.tensor.matmul(ps3[:, 0:M], w3a_sb[:, :], hi2[:, :])
                nc.tensor.matmul(ps3[:, M : 2 * M], w3b_sb[:, :], hi2[:, :])
                nc.vector.tensor_scalar(ob[:, :], ps3[:, :], b3_ap, 0.0,
                                        add, vmax)
                nc.sync.dma_start(out_d[:, 2 * c0 : 2 * c1], ob[:, :])

    nc.compile()
    return nc


def _get_nc(iters):
    if iters not in _NC_CACHE:
        _NC_CACHE[iters] = _build_nc(iters)
    return _NC_CACHE[iters]


def _fold_bn(W, b, gamma, beta, mean, var):
    inv = gamma.astype(np.float64) / np.sqrt(var.astype(np.float64) + EPS)
    Wp = (W.astype(np.float64) * inv[:, None]).astype(np.float32)
    bp = ((b.astype(np.float64) - mean.astype(np.float64)) * inv
          + beta.astype(np.float64)).astype(np.float32)
    return Wp, bp


def _prepare(inputs):
    gp = np.asarray(inputs["grouped_pc"], dtype=np.float32)
    valid = np.asarray(inputs["valid"], dtype=np.float32)

    Wp1, bp1 = _fold_bn(*(np.asarray(inputs[k], dtype=np.float32)
                          for k in ("W1", "b1", "gamma1", "beta1", "mean1", "var1")))
    Wp2, bp2 = _fold_bn(*(np.asarray(inputs[k], dtype=np.float32)
                          for k in ("W2", "b2", "gamma2", "beta2", "mean2", "var2")))
    Wp3, bp3 = _fold_bn(*(np.asarray(inputs[k], dtype=np.float32)
                          for k in ("W3", "b3", "gamma3", "beta3", "mean3", "var3")))

    lhsT1 = np.zeros((128, 128), np.float16)
    lhsT1[0:3, 0:64] = Wp1.T
    lhsT1[3:6, 64:128] = Wp1.T

    lhsT2 = np.zeros((128, 128), np.float16)
    lhsT2[0:64, 0:64] = Wp2.T
    lhsT2[64:128, 64:128] = Wp2.T

    lhsT3a = np.zeros((128, 128), np.float16)
    lhsT3a[0:64, :] = Wp3.T
    lhsT3b = np.zeros((128, 128), np.float16)
    lhsT3b[64:128, :] = Wp3.T

    biases = np.zeros((128, 3), np.float32)
    biases[:, 0] = np.concatenate([bp1, bp1])
    biases[:, 1] = np.concatenate([bp2, bp2])
    biases[:, 2] = bp3

    x = gp[0].reshape(3, NCOLS)
    vidx = np.flatnonzero(valid.reshape(NCOLS) > 0.5)
    V = len(vidx)
    Vc = -(-V // N_CORES)
    iters = max(1, -(-Vc // (2 * M)))
    cap = iters * 2 * M

    xv = x[:, vidx].astype(np.float16)

    in_maps = []
    for c in range(N_CORES):
        lo_i = c * Vc
        hi_i = min((c + 1) * Vc, V)
        n = max(0, hi_i - lo_i)
        a = np.zeros((3, cap), np.float16)
        if n:
            a[:, :n] = xv[:, lo_i:hi_i]
        ar = a.reshape(3, iters, 2, M)
        xp = np.empty((6, iters * M), np.float16)
        xp[0:3] = ar[:, :, 0, :].reshape(3, -1)
        xp[3:6] = ar[:, :, 1, :].reshape(3, -1)
        in_maps.append(
            {
                "xp": np.ascontiguousarray(xp),
                "lhsT1": lhsT1,
                "lhsT2": lhsT2,
                "lhsT3a": lhsT3a,
                "lhsT3b": lhsT3b,
                "biases": biases,
            }
        )
    return in_maps, vidx, V, Vc, iters


def _gather(results, vidx, V, Vc):
    stream = np.empty((128, V), np.float32)
    for c in range(N_CORES):
        lo_i = c * Vc
        hi_i = min((c + 1) * Vc, V)
        if hi_i <= lo_i:
            break
        stream[:, lo_i:hi_i] = results[c]["out"][:, : hi_i - lo_i]
    full = np.zeros((128, NCOLS), np.float32)
    full[:, vidx] = stream
    return full.reshape(128, NPOINT, KNN)[None]


def run_traced(trace=False, **inputs):
    in_maps, vidx, V, Vc, iters = _prepare(inputs)
    nc = _get_nc(iters)
    res = run_bass_kernel_spmd(nc, in_maps, list(range(N_CORES)), trace=trace)
    return _gather(res.results, vidx, V, Vc), res.exec_time_ns


def kernel(**inputs):
    out, _ = run_traced(trace=False, **inputs)
    return out
